# revision 28
# baseline (speedup 1.0000x reference)
"""Trainium2 Bass kernel for nn_DPSpikingDecoder.

Math: the leaky-integrator scan v_t = 0.5*v_{t-1} + x_t, the mean over
channels C, and the differential window pooling are all linear maps over
the time axis, and the scan kernel is identical for every channel.  So

    dp[b, w, f] = sum_{c,t} (K[w, t] / C) * spikes[b, c, t, f]

where K = M_pool @ L_scan is a [W=40, T=960] matrix precomputed on host.
Viewing spikes[b] as a flat [C*T, F] matrix, this is one 30720-long
matmul contraction per sample, streamed through the PE while spikes
stream from HBM exactly once (memory-bound; the per-core HBM roofline
is ~358 GB/s).  float32r gives the full-rate PE path on unmodified
fp32 bytes.

DMA shape (the part that matters): SDMA engine 15 runs ~15-25% slower
than its 15 peers, so with the natural [128, N] transfers (engine =
partition block) its 1/16 byte share accumulates a ~10 us backlog that
stalls the end of the stream while every other engine idles.  Measured
engine-mapping behavior: <=64-partition transfers map partition p ->
engine p//4 and stay at full per-engine rate, so a [60, N] DMA on
partitions 0..59 plus a [60, N] DMA on partitions 64..123 touch
engines 0..14 ONLY.  The stream therefore uses 120-row contraction
chunks living on partitions {0..59, 64..123} (two matmuls per chunk,
one per partition range; partitions 60..63/124..127 are never read),
fed by 16 KB fully-contiguous partition lines.  Engine 15 moves no
stream bytes and the 14 healthy engines run at ~346 GB/s aggregate -
right at the HBM per-core limit - ending in one clean edge.

The tiny MLP + softmax + scale run on-chip as a short tail; layer 1 is
packed 4-wide into PE column groups via tile_position.

Sharding: data-parallel over batch B=8 -> one sample per NeuronCore.
"""

import numpy as np
from contextlib import ExitStack

import concourse.bass as bass
import concourse.bacc as bacc
import concourse.tile as tile
from concourse import mybir
from concourse.bass_utils import run_bass_kernel_spmd

F32 = mybir.dt.float32
F32R = mybir.dt.float32r

B, C, T, F = 8, 32, 960, 256
L_DP, N_DP = 24, 12
W = T // L_DP            # 40 windows
H = 20                   # hidden dim of the MLP

R = C * T                # 30720 contraction rows per sample
CH = 120                 # rows per matmul chunk (partitions 0-59, 64-123)
NCH = R // CH            # 256 chunks
QP = 8                   # weight-tile period: lcm(120, 960) / 120
SW = 16                  # chunks per streamed sub-DMA pair (16 KB lines)
NV = NCH // SW           # 16 sub-DMA pairs; the last arrives in halves


def _host_K():
    """K[w, t] in float64: differential pooling of the decayed scan."""
    t = np.arange(T)
    d = t[:, None] - t[None, :]
    Lmat = np.where(d >= 0, 0.5 ** np.clip(d, 0, None), 0.0)
    M = np.zeros((W, T))
    for w in range(W):
        M[w, w * L_DP + L_DP - N_DP : w * L_DP + L_DP] = 1.0 / N_DP
        M[w, w * L_DP : w * L_DP + N_DP] -= 1.0 / N_DP
    return M @ Lmat  # [W, T]


def _host_kt():
    """SBUF image [128, 2*QP*W]: kt[p, ((h*QP)+q)*W + w] = the weight for
    chunk-row j = 56*h + p, i.e. K[w, (120q + 56h + p)%960]/C.
    Half h=0 covers chunk rows 0..55 ([56, N] DMA), half h=1 rows
    56..119 ([64, N] DMA); both live at partition offset 0 so the PE
    never sees offset operands, and both widths are multiples of 8
    (non-multiple-of-8 partition counts produce corrupted transfers)."""
    K = _host_K()
    img = np.zeros((128, 2 * QP * W), dtype=np.float64)
    for h, wid in ((0, 56), (1, 64)):
        for p in range(wid):
            j = 56 * h + p
            for q in range(QP):
                col = (h * QP + q) * W
                img[p, col : col + W] = K[:, (CH * q + j) % T] / C
    return np.ascontiguousarray(img.astype(np.float32))


def _host_cimg(W2, b2):
    """Packed small consts, one contiguous [128, 101] DMA image:
    cols 0:40 eye(40) on parts 0:40; 40:80 [W2; b2] on parts 0:21;
    col 80 b1 placeholder (zeros, real b1 patched in kernel());
    cols 81:101 the 4-col-group summing matrix."""
    img = np.zeros((128, 101), dtype=np.float32)
    img[0:W, 0:W] = np.eye(W, dtype=np.float32)
    img[0:H, 40:80] = W2.astype(np.float32)
    img[H, 40:80] = b2.astype(np.float32)
    for j in range(4):
        for i in range(H):
            img[32 * j + i, 81 + i] = 1.0
    return img


def _build_program():
    nc = bacc.Bacc(None)
    x = nc.declare_dram_parameter("x", [NV, CH, SW, F], F32R, isOutput=False)
    kt = nc.declare_dram_parameter("kt", [128, 2 * QP * W], F32R, isOutput=False)
    w1r = nc.declare_dram_parameter("w1r", [128, 2 * W * H], F32, isOutput=False)
    cimg = nc.declare_dram_parameter("cimg", [128, 101], F32, isOutput=False)
    y = nc.declare_dram_parameter("y", [W, F], F32, isOutput=True)

    with tile.TileContext(nc) as tc, ExitStack() as ctx:
        consts = ctx.enter_context(tc.tile_pool(name="consts", bufs=1))
        xs = ctx.enter_context(tc.tile_pool(name="xs", bufs=4))
        work = ctx.enter_context(tc.tile_pool(name="work", bufs=1))
        dp_psp = ctx.enter_context(tc.tile_pool(name="dp_ps", bufs=1, space="PSUM"))
        sm_ps = ctx.enter_context(tc.tile_pool(name="sm_ps", bufs=1, space="PSUM"))

        # kt first on the sync ring (the PE needs it for the first MM);
        # cimg on the scalar ring; both are tiny and contiguous.
        kt_sb = consts.tile([128, 2, QP, W], F32R)
        nc.sync.dma_start(
            out=kt_sb, in_=kt[:].rearrange("p (h q w) -> p h q w", h=2, q=QP)
        )
        ci_sb = consts.tile([128, 101], F32)
        nc.scalar.dma_start(out=ci_sb, in_=cimg[:])
        eye_sb = ci_sb[0:W, 0:W]
        w2b_sb = ci_sb[0 : H + 1, 40:80]
        b1_sb = ci_sb[0:H, 80:81]
        sel_sb = ci_sb[:, 81:101]
        w1_sb = consts.tile([128, 2 * W * H], F32)

        # augmented MLP input [h; 1] so layer 2 adds b2 inside the matmul
        h_aug = work.tile([H + 1, 1], F32)
        nc.vector.memset(h_aug, 1.0)  # row H stays 1; rows 0..H-1 overwritten

        # ---- big streamed contraction: dp[w, f] += kt_q^T @ x_chunk ----
        # Each sub-DMA pair: [60, SW, F] on partitions 0..59 (one ring) +
        # [60, SW, F] on partitions 64..123 (other ring) -> engines 0..14.
        dp_ps = dp_psp.tile([W, F], F32)

        def stream_piece(v, s0, ns, ea, eb, tag="m", bufs=3):
            xta = xs.tile([56, ns, F], F32R, tag=f"xta_{tag}", bufs=bufs)
            xtb = xs.tile([64, ns, F], F32R, tag=f"xtb_{tag}", bufs=bufs)
            ea.dma_start(out=xta, in_=x[v, 0:56, s0 : s0 + ns, :])
            eb.dma_start(out=xtb, in_=x[v, 56:120, s0 : s0 + ns, :])
            for s in range(ns):
                m = v * SW + s0 + s
                q = m % QP
                nc.tensor.matmul(
                    dp_ps,
                    lhsT=kt_sb[0:56, 0, q, :],
                    rhs=xta[:, s, :],
                    start=(m == 0),
                    stop=False,
                )
                nc.tensor.matmul(
                    dp_ps,
                    lhsT=kt_sb[0:64, 1, q, :],
                    rhs=xtb[:, s, :],
                    start=False,
                    stop=(m == NCH - 1),
                )

        for v in range(NV):
            ea = nc.sync if v % 2 == 0 else nc.scalar
            eb = nc.scalar if v % 2 == 0 else nc.sync
            if v == NV - 1:
                # final pair in halves so the last matmuls drain sooner
                stream_piece(v, 0, SW // 2, ea, eb, tag="f0", bufs=1)
                stream_piece(v, SW // 2, SW // 2, eb, ea, tag="f1", bufs=1)
            else:
                stream_piece(v, 0, SW, ea, eb)
            # w1 rides mid-stream so it is resident long before the tail
            if v == 8:
                nc.sync.dma_start(out=w1_sb[:, 0 : W * H], in_=w1r[:, 0 : W * H])
                nc.scalar.dma_start(out=w1_sb[:, W * H :], in_=w1r[:, W * H :])

        dp_sb = work.tile([W, F], F32)
        nc.vector.tensor_copy(dp_sb, dp_ps)

        # ---- transpose dp to feed the MLP contraction ----
        dpT_ps = sm_ps.tile([128, 2, W], F32)
        for e in range(2):
            nc.tensor.transpose(dpT_ps[:, e, :], dp_sb[:, e * 128 : (e + 1) * 128], eye_sb)
        dpT_sb = work.tile([128, 2, W], F32)
        nc.vector.tensor_copy(dpT_sb, dpT_ps)

        # ---- layer 1: h = relu(dp_flat @ W1 + b1), 80 chunks of 128 ----
        # packed 4-wide into PE column groups; partial sums land in four
        # partition slices of hp_ps and are summed by one sel-matmul.
        hp_ps = sm_ps.tile([128, 1], F32)
        for m in range(2 * W):
            w, e = divmod(m, 2)
            j = m % 4
            nc.tensor.matmul(
                hp_ps[32 * j : 32 * j + H, :],
                lhsT=w1_sb[:, m * H : (m + 1) * H],
                rhs=dpT_sb[:, e, w : w + 1],
                start=(m < 4),
                stop=(m >= 2 * W - 4),
                tile_position=(0, 32 * j),
            )
        hp_sb = work.tile([128, 1], F32)
        nc.vector.tensor_copy(hp_sb, hp_ps)
        h_ps = sm_ps.tile([H, 1], F32)
        nc.tensor.matmul(h_ps, lhsT=sel_sb, rhs=hp_sb, start=True, stop=True)
        nc.scalar.activation(
            h_aug[0:H, :], h_ps, mybir.ActivationFunctionType.Relu, bias=b1_sb
        )

        # ---- layer 2 (+b2 via augmented row) + softmax on a [1, W] row ----
        a2_ps = sm_ps.tile([1, W], F32)
        nc.tensor.matmul(a2_ps, lhsT=h_aug, rhs=w2b_sb, start=True, stop=True)
        e_sb = work.tile([1, W], F32)
        ssum = work.tile([1, 1], F32)
        nc.scalar.activation(
            e_sb, a2_ps, mybir.ActivationFunctionType.Exp, accum_out=ssum[:]
        )
        rin = work.tile([1, 1], F32)
        nc.vector.reciprocal(rin, ssum)
        ta_sb = work.tile([1, W], F32)
        nc.vector.tensor_scalar_mul(ta_sb, e_sb, rin[:])

        # ---- scale dp rows by attention weights and store ----
        taT_ps = sm_ps.tile([W, 1], F32)
        nc.tensor.transpose(taT_ps, ta_sb, ci_sb[0:1, 0:1])
        ta_col = work.tile([W, 1], F32)
        nc.vector.tensor_copy(ta_col, taT_ps)
        att = work.tile([W, F], F32)
        for e2 in range(2):
            nc.vector.tensor_scalar_mul(
                att[:, e2 * 128 : (e2 + 1) * 128],
                dp_sb[:, e2 * 128 : (e2 + 1) * 128],
                ta_col[:],
            )
        nc.sync.dma_start(out=y[:], in_=att[:])

    nc.compile()
    return nc


_CACHED = {}


def _get_program():
    if "nc" not in _CACHED:
        _CACHED["nc"] = _build_program()
        _CACHED["kt"] = _host_kt()
    return _CACHED["nc"]


def _in_maps(spikes, W1, b1, W2, b2):
    spikes = np.asarray(spikes, dtype=np.float32)
    W1 = np.asarray(W1, dtype=np.float32)
    b1 = np.asarray(b1, dtype=np.float32)
    W2 = np.asarray(W2, dtype=np.float32)
    b2 = np.asarray(b2, dtype=np.float32)
    _get_program()
    # W1 rearranged so chunk m = 2*w + e holds rows d = 256*w + 128*e + p,
    # laid out so the DMA is one contiguous [128, 1600] block.
    w1r = np.ascontiguousarray(
        W1.reshape(W, 2, 128, H).transpose(2, 0, 1, 3).reshape(128, 2 * W * H)
    )
    cimg = _host_cimg(W2, b2)
    cimg[0:H, 80] = b1
    shared = {"kt": _CACHED["kt"], "w1r": w1r, "cimg": cimg}
    # partition-major layout: x[v, p', s, f] = flat[1920 v + 120 s + p', f]
    return [
        {
            "x": np.ascontiguousarray(
                spikes[b].reshape(NV, SW, CH, F).transpose(0, 2, 1, 3)
            ),
            **shared,
        }
        for b in range(B)
    ]


def kernel(spikes, W1, b1, W2, b2):
    in_maps = _in_maps(spikes, W1, b1, W2, b2)
    res = run_bass_kernel_spmd(_get_program(), in_maps, list(range(B)))
    out = np.stack([np.asarray(res.results[i]["y"]).reshape(W * F) for i in range(B)])
    return out.astype(np.float32)


# revision 32
# speedup vs baseline: 1.8389x; 1.8389x over previous
"""Trainium2 Bass kernel for nn_DPSpikingDecoder.

Math: the leaky-integrator scan v_t = 0.5*v_{t-1} + x_t, the mean over
channels C, and the differential window pooling are all linear maps over
the time axis, and the scan kernel is identical for every channel.  So

    dp[b, w, f] = sum_{c,t} (K[w, t] / C) * spikes[b, c, t, f]

where K = M_pool @ L_scan is a [W=40, T=960] matrix precomputed on host.
Viewing spikes[b] as a flat [C*T, F] matrix, this is one 30720-long
matmul contraction per sample, streamed through the PE in 240 chunks of
128 rows while spikes stream from HBM exactly once (memory-bound; the
per-core HBM roofline is ~358 GB/s).  The weight tile for chunk m
depends only on m mod 15 (lcm(128, 960) = 1920 = 15*128), so 15 weight
tiles stay resident in SBUF.  float32r gives the full-rate PE path
(1 cycle/row at N=256) on unmodified fp32 bytes.

HBM layout: the host pre-transposes each sample into partition-major
tiles x[d][p][s][f] = flat[3072 d + 128 s + p, f], so every DMA reads
128 fully-contiguous 24 KB partition lines (one descriptor each) instead
of strided 1 KB elements -- this is the difference between ~22 GB/s and
~27 GiB/s per SDMA engine.  Nine 3 MB tiles alternate between the two
HWDGE rings; the last tile is read as four 768 KB slices so the final
matmuls drain as soon as each lands.  W1 rides mid-stream so the MLP
tail never waits on it.

The tiny MLP + softmax + scale run on-chip as a short tail; layer 1 is
packed 4-wide into PE column groups via tile_position.

Sharding: data-parallel over batch B=8 -> one sample per NeuronCore.
"""

import numpy as np
from contextlib import ExitStack

import concourse.bass as bass
import concourse.bacc as bacc
import concourse.tile as tile
from concourse import mybir
from concourse.bass_utils import run_bass_kernel_spmd

F32 = mybir.dt.float32
F32R = mybir.dt.float32r

B, C, T, F = 8, 32, 960, 256
L_DP, N_DP = 24, 12
W = T // L_DP            # 40 windows
H = 20                   # hidden dim of the MLP

R = C * T                # 30720 contraction rows per sample
CH = 128                 # rows per matmul chunk
NCH = R // CH            # 240 chunks
QP = 15                  # weight-tile period: lcm(128, 960) / 128
CPD = 24                 # chunks per streamed tile (3 MB, one DMA each)
NT = NCH // CPD          # 10 tiles; the last one is read in 4 slices
FQ = CPD // 4            # chunks per final-tile slice


def _host_K():
    """K[w, t] in float64: differential pooling of the decayed scan."""
    t = np.arange(T)
    d = t[:, None] - t[None, :]
    Lmat = np.where(d >= 0, 0.5 ** np.clip(d, 0, None), 0.0)
    M = np.zeros((W, T))
    for w in range(W):
        M[w, w * L_DP + L_DP - N_DP : w * L_DP + L_DP] = 1.0 / N_DP
        M[w, w * L_DP : w * L_DP + N_DP] -= 1.0 / N_DP
    return M @ Lmat  # [W, T]


def _host_kt():
    """SBUF image [CH, QP*W]: kt[p, q*W+w] = K[w, (128q+p)%960]/C."""
    K = _host_K()
    q = np.arange(QP)[:, None]
    p = np.arange(CH)[None, :]
    tidx = (CH * q + p) % T                      # [QP, CH]
    kt2 = K.T[tidx] / C                          # [QP, CH, W]
    img = kt2.transpose(1, 0, 2).reshape(CH, QP * W)
    return np.ascontiguousarray(img.astype(np.float32))


def _host_cimg(W2, b2):
    """Packed small consts, one contiguous [128, 101] DMA image:
    cols 0:40 eye(40) on parts 0:40; 40:80 [W2; b2] on parts 0:21;
    col 80 b1 placeholder (zeros, real b1 patched in kernel());
    cols 81:101 the 4-col-group summing matrix."""
    img = np.zeros((128, 101), dtype=np.float32)
    img[0:W, 0:W] = np.eye(W, dtype=np.float32)
    img[0:H, 40:80] = W2.astype(np.float32)
    img[H, 40:80] = b2.astype(np.float32)
    for j in range(4):
        for i in range(H):
            img[32 * j + i, 81 + i] = 1.0
    return img


def _build_program():
    nc = bacc.Bacc(None)
    x = nc.declare_dram_parameter("x", [NT, CH, CPD, F], F32R, isOutput=False)
    kt = nc.declare_dram_parameter("kt", [CH, QP * W], F32R, isOutput=False)
    w1r = nc.declare_dram_parameter("w1r", [128, 2 * W * H], F32, isOutput=False)
    cimg = nc.declare_dram_parameter("cimg", [128, 101], F32, isOutput=False)
    y = nc.declare_dram_parameter("y", [W, F], F32, isOutput=True)

    with tile.TileContext(nc) as tc, ExitStack() as ctx:
        consts = ctx.enter_context(tc.tile_pool(name="consts", bufs=1))
        xs = ctx.enter_context(tc.tile_pool(name="xs", bufs=3))
        work = ctx.enter_context(tc.tile_pool(name="work", bufs=1))
        dp_psp = ctx.enter_context(tc.tile_pool(name="dp_ps", bufs=1, space="PSUM"))
        sm_ps = ctx.enter_context(tc.tile_pool(name="sm_ps", bufs=1, space="PSUM"))

        # kt/cimg are tiny; their DMAs are emitted after tile 0's so the
        # bulk x descriptors lead the HWDGE generation queue and the SDMA
        # ramp starts earlier.  The PE's first matmul waits for kt anyway.
        kt_sb = consts.tile([CH, QP, W], F32R)
        ci_sb = consts.tile([128, 101], F32)
        eye_sb = ci_sb[0:W, 0:W]
        w2b_sb = ci_sb[0 : H + 1, 40:80]
        b1_sb = ci_sb[0:H, 80:81]
        sel_sb = ci_sb[:, 81:101]
        w1_sb = consts.tile([128, 2 * W * H], F32)

        # augmented MLP input [h; 1] so layer 2 adds b2 inside the matmul
        h_aug = work.tile([H + 1, 1], F32)
        nc.vector.memset(h_aug, 1.0)  # row H stays 1; rows 0..H-1 overwritten

        # ---- big streamed contraction: dp[w, f] += kt_q^T @ x_chunk ----
        # One 3 MB fully-contiguous DMA per tile, alternating rings.
        dp_ps = dp_psp.tile([W, F], F32)
        for d in range(NT - 1):
            xt = xs.tile([CH, CPD, F], F32R)
            eng = nc.sync if d % 2 == 0 else nc.scalar
            eng.dma_start(out=xt, in_=x[d])
            if d == 0:
                nc.sync.dma_start(
                    out=kt_sb, in_=kt[:].rearrange("p (q w) -> p q w", q=QP)
                )
                nc.scalar.dma_start(out=ci_sb, in_=cimg[:])
            # w1 rides mid-stream behind tile 4/5 so it is resident long
            # before the tail, and the final x slices are not delayed.
            if d == 4:
                nc.sync.dma_start(out=w1_sb[:, 0 : W * H], in_=w1r[:, 0 : W * H])
            if d == 5:
                nc.scalar.dma_start(out=w1_sb[:, W * H :], in_=w1r[:, W * H :])
            for s in range(CPD):
                m = d * CPD + s
                nc.tensor.matmul(
                    dp_ps,
                    lhsT=kt_sb[:, m % QP, :],
                    rhs=xt[:, s, :],
                    start=(m == 0),
                    stop=False,
                )
        # last tile arrives as four quarter-DMAs so the final matmuls can
        # drain as soon as each 6-chunk slice lands
        d = NT - 1
        for qd in range(4):
            xt_q = xs.tile([CH, FQ, F], F32R, tag="xt_q", bufs=4)
            eng = nc.sync if qd % 2 == 0 else nc.scalar
            eng.dma_start(out=xt_q, in_=x[d, :, qd * FQ : (qd + 1) * FQ, :])
            for s2 in range(FQ):
                m = d * CPD + qd * FQ + s2
                nc.tensor.matmul(
                    dp_ps,
                    lhsT=kt_sb[:, m % QP, :],
                    rhs=xt_q[:, s2, :],
                    start=False,
                    stop=(m == NCH - 1),
                )

        dp_sb = work.tile([W, F], F32)
        nc.vector.tensor_copy(dp_sb, dp_ps)

        # ---- transpose dp to feed the MLP contraction ----
        dpT_ps = sm_ps.tile([128, 2, W], F32)
        for e in range(2):
            nc.tensor.transpose(dpT_ps[:, e, :], dp_sb[:, e * 128 : (e + 1) * 128], eye_sb)
        dpT_sb = work.tile([128, 2, W], F32)
        nc.vector.tensor_copy(dpT_sb, dpT_ps)

        # ---- layer 1: h = relu(dp_flat @ W1 + b1), 80 chunks of 128 ----
        # operand-flipped: the dpT column is the (1-col, cheap-LDWEIGHTS)
        # stationary operand and the W1 chunk streams, accumulating into a
        # single [1, H] PSUM row -- no cross-column reduction needed.
        h_ps = sm_ps.tile([1, H], F32)
        for m in range(2 * W):
            w, e = divmod(m, 2)
            nc.tensor.matmul(
                h_ps,
                lhsT=dpT_sb[:, e, w : w + 1],
                rhs=w1_sb[:, m * H : (m + 1) * H],
                start=(m == 0),
                stop=(m == 2 * W - 1),
            )
        hrow_sb = work.tile([1, H], F32)
        nc.vector.tensor_copy(hrow_sb, h_ps)
        hT_ps = sm_ps.tile([H, 1], F32)
        nc.tensor.transpose(hT_ps, hrow_sb, ci_sb[0:1, 0:1])
        nc.scalar.activation(
            h_aug[0:H, :], hT_ps, mybir.ActivationFunctionType.Relu, bias=b1_sb
        )

        # ---- layer 2 (+b2 via augmented row) + softmax on a [1, W] row ----
        a2_ps = sm_ps.tile([1, W], F32)
        nc.tensor.matmul(a2_ps, lhsT=h_aug, rhs=w2b_sb, start=True, stop=True)
        e_sb = work.tile([1, W], F32)
        ssum = work.tile([1, 1], F32)
        nc.scalar.activation(
            e_sb, a2_ps, mybir.ActivationFunctionType.Exp, accum_out=ssum[:]
        )
        rin = work.tile([1, 1], F32)
        nc.vector.reciprocal(rin, ssum)
        ta_sb = work.tile([1, W], F32)
        nc.vector.tensor_scalar_mul(ta_sb, e_sb, rin[:])

        # ---- scale dp rows by attention weights and store ----
        taT_ps = sm_ps.tile([W, 1], F32)
        nc.tensor.transpose(taT_ps, ta_sb, ci_sb[0:1, 0:1])
        ta_col = work.tile([W, 1], F32)
        nc.vector.tensor_copy(ta_col, taT_ps)
        att = work.tile([W, F], F32)
        for e2 in range(2):
            nc.vector.tensor_scalar_mul(
                att[:, e2 * 128 : (e2 + 1) * 128],
                dp_sb[:, e2 * 128 : (e2 + 1) * 128],
                ta_col[:],
            )
        nc.sync.dma_start(out=y[:], in_=att[:])

    nc.compile()
    return nc


_CACHED = {}


def _get_program():
    if "nc" not in _CACHED:
        _CACHED["nc"] = _build_program()
        _CACHED["kt"] = _host_kt()
    return _CACHED["nc"]


def _in_maps(spikes, W1, b1, W2, b2):
    spikes = np.asarray(spikes, dtype=np.float32)
    W1 = np.asarray(W1, dtype=np.float32)
    b1 = np.asarray(b1, dtype=np.float32)
    W2 = np.asarray(W2, dtype=np.float32)
    b2 = np.asarray(b2, dtype=np.float32)
    _get_program()
    # W1 rearranged so chunk m = 2*w + e holds rows d = 256*w + 128*e + p,
    # laid out so the DMA is one contiguous [128, 1600] block.
    w1r = np.ascontiguousarray(
        W1.reshape(W, 2, 128, H).transpose(2, 0, 1, 3).reshape(128, 2 * W * H)
    )
    cimg = _host_cimg(W2, b2)
    cimg[0:H, 80] = b1
    shared = {"kt": _CACHED["kt"], "w1r": w1r, "cimg": cimg}
    # partition-major tile layout: x[d, p, s, f] = flat[3072 d + 128 s + p, f]
    return [
        {
            "x": np.ascontiguousarray(
                spikes[b].reshape(NT, CPD, CH, F).transpose(0, 2, 1, 3)
            ),
            **shared,
        }
        for b in range(B)
    ]


def kernel(spikes, W1, b1, W2, b2):
    in_maps = _in_maps(spikes, W1, b1, W2, b2)
    res = run_bass_kernel_spmd(_get_program(), in_maps, list(range(B)))
    out = np.stack([np.asarray(res.results[i]["y"]).reshape(W * F) for i in range(B)])
    return out.astype(np.float32)


# revision 35
# speedup vs baseline: 1.9073x; 1.0372x over previous
"""Trainium2 Bass kernel for nn_DPSpikingDecoder.

Math: the leaky-integrator scan v_t = 0.5*v_{t-1} + x_t, the mean over
channels C, and the differential window pooling are all linear maps over
the time axis, and the scan kernel is identical for every channel.  So

    dp[b, w, f] = sum_{c,t} (K[w, t] / C) * spikes[b, c, t, f]

where K = M_pool @ L_scan is a [W=40, T=960] matrix precomputed on host.
Viewing spikes[b] as a flat [C*T, F] matrix, this is one 30720-long
matmul contraction per sample, streamed through the PE in 240 chunks of
128 rows while spikes stream from HBM exactly once (memory-bound; the
per-core HBM roofline is ~358 GB/s).  The weight tile for chunk m
depends only on m mod 15 (lcm(128, 960) = 1920 = 15*128), so 15 weight
tiles stay resident in SBUF.  float32r gives the full-rate PE path
(1 cycle/row at N=256) on unmodified fp32 bytes.

HBM layout: the host pre-transposes each sample into partition-major
tiles x[d][p][s][f] = flat[3072 d + 128 s + p, f], so every DMA reads
128 fully-contiguous 24 KB partition lines (one descriptor each) instead
of strided 1 KB elements -- this is the difference between ~22 GB/s and
~27 GiB/s per SDMA engine.  Nine 3 MB tiles alternate between the two
HWDGE rings; the last tile is read as four 768 KB slices so the final
matmuls drain as soon as each lands.  W1 rides mid-stream so the MLP
tail never waits on it.

The tiny MLP + softmax + scale run on-chip as a short tail; layer 1 is
packed 4-wide into PE column groups via tile_position.

Sharding: data-parallel over batch B=8 -> one sample per NeuronCore.
"""

import numpy as np
from contextlib import ExitStack

import concourse.bass as bass
import concourse.bacc as bacc
import concourse.tile as tile
from concourse import mybir
from concourse.bass_utils import run_bass_kernel_spmd

F32 = mybir.dt.float32
F32R = mybir.dt.float32r

B, C, T, F = 8, 32, 960, 256
L_DP, N_DP = 24, 12
W = T // L_DP            # 40 windows
H = 20                   # hidden dim of the MLP

R = C * T                # 30720 contraction rows per sample
CH = 128                 # rows per matmul chunk
NCH = R // CH            # 240 chunks
QP = 15                  # weight-tile period: lcm(128, 960) / 128
CPD = 24                 # chunks per streamed tile (3 MB, one DMA each)
NT = NCH // CPD          # 10 tiles; the last one is read in 4 slices
FQ = CPD // 4            # chunks per final-tile slice


def _host_K():
    """K[w, t] in float64: differential pooling of the decayed scan."""
    t = np.arange(T)
    d = t[:, None] - t[None, :]
    Lmat = np.where(d >= 0, 0.5 ** np.clip(d, 0, None), 0.0)
    M = np.zeros((W, T))
    for w in range(W):
        M[w, w * L_DP + L_DP - N_DP : w * L_DP + L_DP] = 1.0 / N_DP
        M[w, w * L_DP : w * L_DP + N_DP] -= 1.0 / N_DP
    return M @ Lmat  # [W, T]


def _host_kt():
    """SBUF image [CH, QP*W]: kt[p, q*W+w] = K[w, (128q+p)%960]/C."""
    K = _host_K()
    q = np.arange(QP)[:, None]
    p = np.arange(CH)[None, :]
    tidx = (CH * q + p) % T                      # [QP, CH]
    kt2 = K.T[tidx] / C                          # [QP, CH, W]
    img = kt2.transpose(1, 0, 2).reshape(CH, QP * W)
    return np.ascontiguousarray(img.astype(np.float32))


def _host_cimg(W2, b2):
    """Packed small consts, one contiguous [128, 101] DMA image:
    cols 0:40 eye(40) on parts 0:40; 40:80 [W2; b2] on parts 0:21;
    col 80 b1 placeholder (zeros, real b1 patched in kernel());
    cols 81:101 the 4-col-group summing matrix."""
    img = np.zeros((128, 101), dtype=np.float32)
    img[0:W, 0:W] = np.eye(W, dtype=np.float32)
    img[0:H, 40:80] = W2.astype(np.float32)
    img[H, 40:80] = b2.astype(np.float32)
    for j in range(4):
        for i in range(H):
            img[32 * j + i, 81 + i] = 1.0
    return img


def _build_program():
    nc = bacc.Bacc(None)
    x = nc.declare_dram_parameter("x", [NT, CH, CPD, F], F32R, isOutput=False)
    kt = nc.declare_dram_parameter("kt", [CH, QP * W], F32R, isOutput=False)
    w1r = nc.declare_dram_parameter("w1r", [128, 2 * W * H], F32, isOutput=False)
    cimg = nc.declare_dram_parameter("cimg", [128, 101], F32, isOutput=False)
    y = nc.declare_dram_parameter("y", [W, F], F32, isOutput=True)

    with tile.TileContext(nc) as tc, ExitStack() as ctx:
        consts = ctx.enter_context(tc.tile_pool(name="consts", bufs=1))
        xs = ctx.enter_context(tc.tile_pool(name="xs", bufs=3))
        work = ctx.enter_context(tc.tile_pool(name="work", bufs=1))
        dp_psp = ctx.enter_context(tc.tile_pool(name="dp_ps", bufs=1, space="PSUM"))
        sm_ps = ctx.enter_context(tc.tile_pool(name="sm_ps", bufs=1, space="PSUM"))

        # kt first on the sync ring (the PE needs it for the first MM);
        # cimg on the scalar ring; both are tiny and contiguous.
        kt_sb = consts.tile([CH, QP, W], F32R)
        nc.sync.dma_start(out=kt_sb, in_=kt[:].rearrange("p (q w) -> p q w", q=QP))
        ci_sb = consts.tile([128, 101], F32)
        nc.scalar.dma_start(out=ci_sb, in_=cimg[:])
        eye_sb = ci_sb[0:W, 0:W]
        w2b_sb = ci_sb[0 : H + 1, 40:80]
        b1_sb = ci_sb[0:H, 80:81]
        sel_sb = ci_sb[:, 81:101]
        w1_sb = consts.tile([128, 2 * W * H], F32)

        # augmented MLP input [h; 1] so layer 2 adds b2 inside the matmul
        h_aug = work.tile([H + 1, 1], F32)
        nc.vector.memset(h_aug, 1.0)  # row H stays 1; rows 0..H-1 overwritten

        # ---- big streamed contraction: dp[w, f] += kt_q^T @ x_chunk ----
        # One 3 MB fully-contiguous DMA per tile, alternating rings.
        dp_ps = dp_psp.tile([W, F], F32)
        for d in range(NT - 1):
            xt = xs.tile([CH, CPD, F], F32R)
            eng = nc.sync if d % 2 == 0 else nc.scalar
            eng.dma_start(out=xt, in_=x[d])
            # w1 rides mid-stream behind tile 4/5 so it is resident long
            # before the tail, and the final x slices are not delayed.
            if d == 4:
                nc.sync.dma_start(out=w1_sb[:, 0 : W * H], in_=w1r[:, 0 : W * H])
            if d == 5:
                nc.scalar.dma_start(out=w1_sb[:, W * H :], in_=w1r[:, W * H :])
            for s in range(CPD):
                m = d * CPD + s
                nc.tensor.matmul(
                    dp_ps,
                    lhsT=kt_sb[:, m % QP, :],
                    rhs=xt[:, s, :],
                    start=(m == 0),
                    stop=False,
                )
        # last tile arrives as four quarter-DMAs so the final matmuls can
        # drain as soon as each 6-chunk slice lands.  SDMA engine 15 runs
        # ~20% slower than its peers and drains a ~10 us backlog here;
        # harmless matmuls on resident kt data keep the PE busy through
        # each wait so HAM never throttles it to the ~2x-slower cold rate
        # for the real trailing matmuls and the MLP tail.
        d = NT - 1
        scr_ps = sm_ps.tile([W, 6 * W], F32)
        for qd in range(4):
            xt_q = xs.tile([CH, FQ, F], F32R, tag="xt_q", bufs=4)
            eng = nc.sync if qd % 2 == 0 else nc.scalar
            eng.dma_start(out=xt_q, in_=x[d, :, qd * FQ : (qd + 1) * FQ, :])
            for u in range(18):
                nc.tensor.matmul(
                    scr_ps,
                    lhsT=kt_sb[:, u % QP, :],
                    rhs=kt_sb[:, 0:6, :],
                    start=True,
                    stop=True,
                )
            for s2 in range(FQ):
                m = d * CPD + qd * FQ + s2
                nc.tensor.matmul(
                    dp_ps,
                    lhsT=kt_sb[:, m % QP, :],
                    rhs=xt_q[:, s2, :],
                    start=False,
                    stop=(m == NCH - 1),
                )

        dp_sb = work.tile([W, F], F32)
        nc.vector.tensor_copy(dp_sb, dp_ps)

        # ---- transpose dp to feed the MLP contraction ----
        dpT_ps = sm_ps.tile([128, 2, W], F32)
        for e in range(2):
            nc.tensor.transpose(dpT_ps[:, e, :], dp_sb[:, e * 128 : (e + 1) * 128], eye_sb)
        dpT_sb = work.tile([128, 2, W], F32)
        nc.vector.tensor_copy(dpT_sb, dpT_ps)

        # ---- layer 1: h = relu(dp_flat @ W1 + b1), 80 chunks of 128 ----
        # packed 4-wide into PE column groups; partial sums land in four
        # partition slices of hp_ps and are summed by one sel-matmul.
        hp_ps = sm_ps.tile([128, 1], F32)
        for m in range(2 * W):
            w, e = divmod(m, 2)
            j = m % 4
            nc.tensor.matmul(
                hp_ps[32 * j : 32 * j + H, :],
                lhsT=w1_sb[:, m * H : (m + 1) * H],
                rhs=dpT_sb[:, e, w : w + 1],
                start=(m < 4),
                stop=(m >= 2 * W - 4),
                tile_position=(0, 32 * j),
            )
        hp_sb = work.tile([128, 1], F32)
        nc.vector.tensor_copy(hp_sb, hp_ps)
        h_ps = sm_ps.tile([H, 1], F32)
        nc.tensor.matmul(h_ps, lhsT=sel_sb, rhs=hp_sb, start=True, stop=True)
        nc.scalar.activation(
            h_aug[0:H, :], h_ps, mybir.ActivationFunctionType.Relu, bias=b1_sb
        )

        # ---- layer 2 (+b2 via augmented row) + softmax on a [1, W] row ----
        a2_ps = sm_ps.tile([1, W], F32)
        nc.tensor.matmul(a2_ps, lhsT=h_aug, rhs=w2b_sb, start=True, stop=True)
        e_sb = work.tile([1, W], F32)
        ssum = work.tile([1, 1], F32)
        nc.scalar.activation(
            e_sb, a2_ps, mybir.ActivationFunctionType.Exp, accum_out=ssum[:]
        )
        rin = work.tile([1, 1], F32)
        nc.vector.reciprocal(rin, ssum)
        ta_sb = work.tile([1, W], F32)
        nc.vector.tensor_scalar_mul(ta_sb, e_sb, rin[:])

        # ---- scale dp rows by attention weights and store ----
        taT_ps = sm_ps.tile([W, 1], F32)
        nc.tensor.transpose(taT_ps, ta_sb, ci_sb[0:1, 0:1])
        ta_col = work.tile([W, 1], F32)
        nc.vector.tensor_copy(ta_col, taT_ps)
        att = work.tile([W, F], F32)
        for e2 in range(2):
            nc.vector.tensor_scalar_mul(
                att[:, e2 * 128 : (e2 + 1) * 128],
                dp_sb[:, e2 * 128 : (e2 + 1) * 128],
                ta_col[:],
            )
        nc.sync.dma_start(out=y[:], in_=att[:])

    nc.compile()
    return nc


_CACHED = {}


def _get_program():
    if "nc" not in _CACHED:
        _CACHED["nc"] = _build_program()
        _CACHED["kt"] = _host_kt()
    return _CACHED["nc"]


def _in_maps(spikes, W1, b1, W2, b2):
    spikes = np.asarray(spikes, dtype=np.float32)
    W1 = np.asarray(W1, dtype=np.float32)
    b1 = np.asarray(b1, dtype=np.float32)
    W2 = np.asarray(W2, dtype=np.float32)
    b2 = np.asarray(b2, dtype=np.float32)
    _get_program()
    # W1 rearranged so chunk m = 2*w + e holds rows d = 256*w + 128*e + p,
    # laid out so the DMA is one contiguous [128, 1600] block.
    w1r = np.ascontiguousarray(
        W1.reshape(W, 2, 128, H).transpose(2, 0, 1, 3).reshape(128, 2 * W * H)
    )
    cimg = _host_cimg(W2, b2)
    cimg[0:H, 80] = b1
    shared = {"kt": _CACHED["kt"], "w1r": w1r, "cimg": cimg}
    # partition-major tile layout: x[d, p, s, f] = flat[3072 d + 128 s + p, f]
    return [
        {
            "x": np.ascontiguousarray(
                spikes[b].reshape(NT, CPD, CH, F).transpose(0, 2, 1, 3)
            ),
            **shared,
        }
        for b in range(B)
    ]


def kernel(spikes, W1, b1, W2, b2):
    in_maps = _in_maps(spikes, W1, b1, W2, b2)
    res = run_bass_kernel_spmd(_get_program(), in_maps, list(range(B)))
    out = np.stack([np.asarray(res.results[i]["y"]).reshape(W * F) for i in range(B)])
    return out.astype(np.float32)


# revision 38
# speedup vs baseline: 2.1386x; 1.1213x over previous
"""Trainium2 Bass kernel for nn_DPSpikingDecoder — streaming-tail variant.

Same math as v5a (one long PE contraction against a host-precomputed
scan+pool kernel), but rows are ordered time-major: sub-DMA w holds
exactly window w ([128, 6, F], 6 KB partition lines).  Because the scan
kernel is shift-invariant (K[w, t] = f(24w - t), with contributions
beyond the previous window < 2^-25), window w's dp row is final once
sub-DMAs w-1 and w are matmul'd.  Each group of 5 windows therefore
retires mid-stream: its PSUM rows are copied out, transposed, and its
10 MLP-layer-1 matmuls run in the shadow of the DMA stream, leaving
only ~1 window of contraction + the tiny layer-2/softmax/scale tail
after the last byte lands.  W1 streams in 8 per-group slices, each a
group ahead of its first use, so the in-order PE queue never blocks.

Sharding: data-parallel over batch B=8 -> one sample per NeuronCore.
"""

import numpy as np
from contextlib import ExitStack

import concourse.bass as bass
import concourse.bacc as bacc
import concourse.tile as tile
from concourse import mybir
from concourse.bass_utils import run_bass_kernel_spmd

F32 = mybir.dt.float32
F32R = mybir.dt.float32r

B, C, T, F = 8, 32, 960, 256
L_DP, N_DP = 24, 12
W = T // L_DP            # 40 windows
H = 20                   # hidden dim of the MLP

R = C * T                # 30720 contraction rows per sample
CH = 128                 # rows per matmul chunk (= 4 time steps x 32 ch)
SW = 6                   # chunks per sub-DMA = one 24-step window
NW = W                   # 40 sub-DMAs
G = 5                    # windows per PSUM group
NG = W // G              # 8 groups


def _host_K():
    """K[w, t] in float64: differential pooling of the decayed scan."""
    t = np.arange(T)
    d = t[:, None] - t[None, :]
    Lmat = np.where(d >= 0, 0.5 ** np.clip(d, 0, None), 0.0)
    M = np.zeros((W, T))
    for w in range(W):
        M[w, w * L_DP + L_DP - N_DP : w * L_DP + L_DP] = 1.0 / N_DP
        M[w, w * L_DP : w * L_DP + N_DP] -= 1.0 / N_DP
    return M @ Lmat  # [W, T]


def _host_kt():
    """Shift-invariant kernel images, one [128, (5*6 + 6)*5] block:
    mains  ktm[p, (r*6+k)*5 + c] = f(24(c-r) - tloc)/C,  tloc = 4k + p//32
    tails  ktt[p, (30+k)*5 + 0]  = f(24 - tloc)/C  (cols 1..4 zero)
    where f(gap) = K[w, 24w - gap] for any deep w (shift invariance)."""
    K = _host_K()
    f = np.zeros(512)  # f[gap + 256]
    for gap in range(-119, 121):
        f[gap + 256] = K[20, 480 - gap]
    p = np.arange(128)
    tloc = lambda k: 4 * k + p // 32          # [128]
    img = np.zeros((128, (G * SW + SW) * G), dtype=np.float64)
    for r in range(G):
        for k in range(SW):
            for c in range(G):
                img[:, (r * SW + k) * G + c] = f[24 * (c - r) - tloc(k) + 256] / C
    for k in range(SW):
        img[:, (G * SW + k) * G + 0] = f[24 - tloc(k) + 256] / C
    return np.ascontiguousarray(img.astype(np.float32))


def _host_cimg(W2, b2):
    """Packed small consts, one contiguous [128, 101] DMA image:
    cols 0:40 eye(40) on parts 0:40; 40:80 [W2; b2] on parts 0:21;
    col 80 b1 placeholder (zeros, real b1 patched in kernel());
    cols 81:101 the 4-col-group summing matrix."""
    img = np.zeros((128, 101), dtype=np.float32)
    img[0:W, 0:W] = np.eye(W, dtype=np.float32)
    img[0:H, 40:80] = W2.astype(np.float32)
    img[H, 40:80] = b2.astype(np.float32)
    for j in range(4):
        for i in range(H):
            img[32 * j + i, 81 + i] = 1.0
    return img


def _build_program():
    nc = bacc.Bacc(None)
    x = nc.declare_dram_parameter("x", [NW, CH, SW, F], F32R, isOutput=False)
    kt = nc.declare_dram_parameter("kt", [128, (G * SW + SW) * G], F32R, isOutput=False)
    w1r = nc.declare_dram_parameter("w1r", [128, 2 * W * H], F32, isOutput=False)
    cimg = nc.declare_dram_parameter("cimg", [128, 101], F32, isOutput=False)
    y = nc.declare_dram_parameter("y", [W, F], F32, isOutput=True)

    with tile.TileContext(nc) as tc, ExitStack() as ctx:
        consts = ctx.enter_context(tc.tile_pool(name="consts", bufs=1))
        xs = ctx.enter_context(tc.tile_pool(name="xs", bufs=8))
        work = ctx.enter_context(tc.tile_pool(name="work", bufs=1))
        g_psp = ctx.enter_context(tc.tile_pool(name="g_ps", bufs=2, space="PSUM"))
        dpt_psp = ctx.enter_context(tc.tile_pool(name="dpt_ps", bufs=2, space="PSUM"))
        hp_psp = ctx.enter_context(tc.tile_pool(name="hp_ps", bufs=1, space="PSUM"))
        sm_ps = ctx.enter_context(tc.tile_pool(name="sm_ps", bufs=1, space="PSUM"))

        kt_sb = consts.tile([128, G * SW + SW, G], F32R)
        nc.sync.dma_start(
            out=kt_sb, in_=kt[:].rearrange("p (q c) -> p q c", c=G)
        )
        ci_sb = consts.tile([128, 101], F32)
        nc.scalar.dma_start(out=ci_sb, in_=cimg[:])
        eye5 = ci_sb[0:G, 0:G]
        w2b_sb = ci_sb[0 : H + 1, 40:80]
        b1_sb = ci_sb[0:H, 80:81]
        sel_sb = ci_sb[:, 81:101]
        w1_sb = consts.tile([128, 2 * W * H], F32)

        h_aug = work.tile([H + 1, 1], F32)
        nc.vector.memset(h_aug, 1.0)
        dp_sb = work.tile([W, F], F32)
        dpT_sb = work.tile([128, 2, W], F32)
        hp_ps = hp_psp.tile([128, 1], F32)

        g_tiles = [None] * NG
        xt_prev = None

        for w in range(NW):
            g, r = divmod(w, G)
            xt = xs.tile([CH, SW, F], F32R)
            eng = nc.sync if w % 2 == 0 else nc.scalar
            eng.dma_start(out=xt, in_=x[w])
            # stream W1 in per-group slices, one group ahead of use
            if r == 0:
                for sg in ([0, 1] if g == 0 else [g + 1]):
                    if sg < NG:
                        oeng = nc.scalar if w % 2 == 0 else nc.sync
                        oeng.dma_start(
                            out=w1_sb[:, sg * 2 * G * H : (sg + 1) * 2 * G * H],
                            in_=w1r[:, sg * 2 * G * H : (sg + 1) * 2 * G * H],
                        )
            if r == 0:
                gt_new = g_psp.tile([G, F], F32)
                g_tiles[g] = gt_new
                if g > 0:
                    # decay tails of window 5g-1 into G_g row 0; the first
                    # is full-width with start=True so it initializes all
                    # 5 rows (cols 1..4 of ktt are zero)
                    for k in range(SW):
                        nc.tensor.matmul(
                            g_tiles[g],
                            lhsT=kt_sb[:, G * SW + k, :],
                            rhs=xt_prev[:, k, :],
                            start=(k == 0),
                            stop=False,
                        )
            for k in range(SW):
                nc.tensor.matmul(
                    g_tiles[g],
                    lhsT=kt_sb[:, r * SW + k, :],
                    rhs=xt[:, k, :],
                    start=(w == 0 and k == 0),
                    stop=(r == G - 1 and k == SW - 1),
                )
            xt_prev = xt
            if r == G - 1:
                # group g is final: retire it in the stream's shadow.
                # DVE/PE ops need base partition 0, so stage the 5 rows at
                # partition 0 and assemble dp_sb via a tiny SBUF->SBUF DMA
                # (DMAs have no base-partition restriction).
                stage = work.tile([G, F], F32, tag="stage", bufs=2)
                nc.vector.tensor_copy(stage, g_tiles[g])
                nc.scalar.dma_start(out=dp_sb[G * g : G * (g + 1), :], in_=stage)
                dpT_ps = dpt_psp.tile([128, 2, G], F32)
                for e in range(2):
                    nc.tensor.transpose(
                        dpT_ps[:, e, :],
                        stage[:, e * 128 : (e + 1) * 128],
                        eye5,
                    )
                nc.vector.tensor_copy(dpT_sb[:, :, G * g : G * (g + 1)], dpT_ps)
                for m in range(2 * G * g, 2 * G * (g + 1)):
                    wi, e = divmod(m, 2)
                    j = m % 4
                    nc.tensor.matmul(
                        hp_ps[32 * j : 32 * j + H, :],
                        lhsT=w1_sb[:, m * H : (m + 1) * H],
                        rhs=dpT_sb[:, e, wi : wi + 1],
                        start=(m < 4),
                        stop=(m >= 2 * W - 4),
                        tile_position=(0, 32 * j),
                    )

        # ---- tiny MLP tail: only layer 2 + softmax + scale remain ----
        hp_sb = work.tile([128, 1], F32)
        nc.vector.tensor_copy(hp_sb, hp_ps)
        h_ps = sm_ps.tile([H, 1], F32)
        nc.tensor.matmul(h_ps, lhsT=sel_sb, rhs=hp_sb, start=True, stop=True)
        nc.scalar.activation(
            h_aug[0:H, :], h_ps, mybir.ActivationFunctionType.Relu, bias=b1_sb
        )
        a2_ps = sm_ps.tile([1, W], F32)
        nc.tensor.matmul(a2_ps, lhsT=h_aug, rhs=w2b_sb, start=True, stop=True)
        e_sb = work.tile([1, W], F32)
        ssum = work.tile([1, 1], F32)
        nc.scalar.activation(
            e_sb, a2_ps, mybir.ActivationFunctionType.Exp, accum_out=ssum[:]
        )
        rin = work.tile([1, 1], F32)
        nc.vector.reciprocal(rin, ssum)
        ta_sb = work.tile([1, W], F32)
        nc.vector.tensor_scalar_mul(ta_sb, e_sb, rin[:])
        taT_ps = sm_ps.tile([W, 1], F32)
        nc.tensor.transpose(taT_ps, ta_sb, ci_sb[0:1, 0:1])
        ta_col = work.tile([W, 1], F32)
        nc.vector.tensor_copy(ta_col, taT_ps)
        att = work.tile([W, F], F32)
        for e2 in range(2):
            nc.vector.tensor_scalar_mul(
                att[:, e2 * 128 : (e2 + 1) * 128],
                dp_sb[:, e2 * 128 : (e2 + 1) * 128],
                ta_col[:],
            )
        nc.sync.dma_start(out=y[:], in_=att[:])

    nc.compile()
    return nc


_CACHED = {}


def _get_program():
    if "nc" not in _CACHED:
        _CACHED["nc"] = _build_program()
        _CACHED["kt"] = _host_kt()
    return _CACHED["nc"]


def _in_maps(spikes, W1, b1, W2, b2):
    spikes = np.asarray(spikes, dtype=np.float32)
    W1 = np.asarray(W1, dtype=np.float32)
    b1 = np.asarray(b1, dtype=np.float32)
    W2 = np.asarray(W2, dtype=np.float32)
    b2 = np.asarray(b2, dtype=np.float32)
    _get_program()
    w1r = np.ascontiguousarray(
        W1.reshape(W, 2, 128, H).transpose(2, 0, 1, 3).reshape(128, 2 * W * H)
    )
    cimg = _host_cimg(W2, b2)
    cimg[0:H, 80] = b1
    shared = {"kt": _CACHED["kt"], "w1r": w1r, "cimg": cimg}
    # time-major partition layout:
    #   x[w, 32*pp + c, k, f] = spikes[b, c, 24w + 4k + pp, f]
    maps = []
    for b in range(B):
        tcf = spikes[b].transpose(1, 0, 2)                 # [T, C, F]
        x_ = np.ascontiguousarray(
            tcf.reshape(NW, SW, 4, C, F).transpose(0, 2, 3, 1, 4).reshape(NW, CH, SW, F)
        )
        maps.append({"x": x_, **shared})
    return maps


def kernel(spikes, W1, b1, W2, b2):
    in_maps = _in_maps(spikes, W1, b1, W2, b2)
    res = run_bass_kernel_spmd(_get_program(), in_maps, list(range(B)))
    out = np.stack([np.asarray(res.results[i]["y"]).reshape(W * F) for i in range(B)])
    return out.astype(np.float32)
